# revision 4
# baseline (speedup 1.0000x reference)
"""ColWarp (per-sample color warp + shift + depthwise 5x5 conv) on 8 TRN2 cores.

Decomposition: out[c] = conv5x5(sum_d W[d,c]*(im[d]+shift[d]), k) is linear,
so the 3x3 color warp, the per-channel shift, and the 5-tap column conv all
fold into per-sample banded Toeplitz stationary matrices built on host from
flat_col (32x37).  Each output row-tile is 5 PSUM-accumulated fp32r matmuls
(one per kernel column dx, the dx shift expressed as a free-dim offset into a
zero-padded rhs tile).  The shift term rides on a constant ones row at
partition 126 whose stationary coefficient is s'[c] * (sum of dy-valid kernel
taps), which reproduces the zero-padding border behavior exactly.

Data parallel: 4 samples per core, 8 cores, no cross-core communication.
"""

import numpy as np

import concourse.bass as bass
import concourse.mybir as mybir
import concourse.tile as tile
from concourse.bass_utils import run_bass_kernel_spmd

BS, C, H, W = 32, 3, 768, 768
NCORES = 8
SPC = BS // NCORES  # 4 samples per core

ROWS = 38  # output rows per h-tile
WIN = ROWS + 4  # input window rows per channel (42)
NT = -(-H // ROWS)  # 21 h-tiles (last has 8 rows)
KC = 3 * WIN + 1  # matmul contraction: 126 image rows + ones row
ONES = 3 * WIN  # partition index of the ones row (126)
FREE = W + 4  # rhs width incl. 2+2 zero side cols (772)
HPAD = 802  # padded height: 2 + 768 + 32 (t20 window slack)
NCLS = 3  # stationary classes: first / interior / last tile
NSTAT = NCLS * 5  # stationaries per sample
STATW = SPC * NSTAT * 128  # stat tensor free width per core

_nc_cache = {}


def _tile_geom(t):
    a = t * ROWS
    rows = min(ROWS, H - a)
    cls = 0 if t == 0 else (2 if t == NT - 1 else 1)
    return a, rows, cls


def _legalize_waits(nc):
    # This walrus build rejects >1 sync wait per instruction; move extra
    # waits onto same-engine NOPs immediately before (sequencers execute
    # waits in program order, so this is equivalent).
    for f in nc.m.functions:
        for blk in f.blocks:
            out = []
            changed = False
            for inst in blk.instructions:
                si = inst.sync_info
                waits = list(si.on_wait) if si is not None and si.on_wait else []
                if len(waits) > 1:
                    changed = True
                    for j, w in enumerate(waits[:-1]):
                        out.append(
                            mybir.InstNoOp(
                                name=f"{inst.name}-wsplit{j}",
                                engine=inst.engine,
                                ins=[],
                                outs=[],
                                sync_info=mybir.SyncInfo(on_wait=[w], on_update=[]),
                            )
                        )
                    inst.sync_info = mybir.SyncInfo(
                        on_wait=[waits[-1]],
                        on_update=list(si.on_update) if si.on_update else [],
                    )
                out.append(inst)
            if changed:
                blk.instructions = out


def _build_program():
    f32 = mybir.dt.float32
    f32r = mybir.dt.float32r
    nc = bass.Bass()
    im_dram = nc.declare_dram_parameter(
        "im_pad", [SPC, C, HPAD, FREE], f32r, isOutput=False
    )
    stat_dram = nc.declare_dram_parameter("stat", [128, STATW], f32r, isOutput=False)
    ones_dram = nc.declare_dram_parameter("ones_row", [1, FREE], f32r, isOutput=False)
    out_dram = nc.declare_dram_parameter("out", [SPC, C, H, W], f32, isOutput=True)

    with tile.TileContext(nc) as tc:
        with (
            tc.tile_pool(name="stat", bufs=1) as stat_pool,
            tc.tile_pool(name="rhs", bufs=4) as rhs_pool,
            tc.tile_pool(name="outb", bufs=4) as out_pool,
            tc.tile_pool(name="psum", bufs=3, space="PSUM") as psum_pool,
        ):
            stat_t = stat_pool.tile([128, STATW], f32r)
            per_b = NSTAT * 128
            for b in range(SPC):
                nc.sync.dma_start(
                    out=stat_t[:, b * per_b : (b + 1) * per_b],
                    in_=stat_dram[:, b * per_b : (b + 1) * per_b],
                )

            n_tile = 0
            for b in range(SPC):
                for t in range(NT):
                    a, rows, cls = _tile_geom(t)
                    mt = 3 * rows

                    rhs_t = rhs_pool.tile([128, FREE], f32r, tag="rhs")
                    # window rows [a-2, a+40) of the original image =
                    # rows [a, a+42) of the host-padded image
                    nc.sync.dma_start(
                        out=rhs_t[0 : 3 * WIN, :],
                        in_=im_dram[b, :, a : a + WIN, :],
                    )
                    nc.sync.dma_start(out=rhs_t[ONES : ONES + 1, :], in_=ones_dram[:])

                    psum_t = psum_pool.tile([114, W], f32, tag="ps")
                    for cs, nsz in ((0, 512), (512, 256)):
                        for dxi in range(5):
                            idx = (b * NCLS + cls) * 5 + dxi
                            nc.tensor.matmul(
                                psum_t[:mt, cs : cs + nsz],
                                stat_t[:KC, idx * 128 : idx * 128 + mt],
                                rhs_t[:KC, cs + dxi : cs + dxi + nsz],
                                start=(dxi == 0),
                                stop=(dxi == 4),
                            )

                    out_t = out_pool.tile([114, W], f32, tag="ob")
                    if n_tile % 2 == 0:
                        nc.vector.tensor_copy(out=out_t[:mt, :], in_=psum_t[:mt, :])
                    else:
                        nc.scalar.copy(out=out_t[:mt, :], in_=psum_t[:mt, :])
                    nc.scalar.dma_start(
                        out=out_dram[b, :, a : a + rows, :],
                        in_=out_t[:mt, :],
                    )
                    n_tile += 1

    _legalize_waits(nc)
    return nc


def _get_program():
    if "nc" not in _nc_cache:
        _nc_cache["nc"] = _build_program()
    return _nc_cache["nc"]


def _build_stats(flat_col):
    """flat_col [BS, 37] float32 -> stats [BS, NCLS, 5, 128, 128] float32."""
    flat_col = np.asarray(flat_col, np.float64)
    nb = flat_col.shape[0]
    geoms = [_tile_geom(t) for t in (0, 1, NT - 1)]

    # index arrays per class (independent of sample and dx)
    cls_idx = []
    for a, rows, cls in geoms:
        base = a - 2
        ks, ms, ds, cs, dys = [], [], [], [], []
        ones_m = []
        ones_c = []
        ones_mask = np.zeros((3 * rows, 5), np.float64)
        for c in range(3):
            for i in range(rows):
                m = c * rows + i
                h = a + i
                for dy in range(-2, 3):
                    h2 = h + dy
                    if 0 <= h2 < H:
                        ones_mask[m, dy + 2] = 1.0
                        for d in range(3):
                            ks.append(d * WIN + (h2 - base))
                            ms.append(m)
                            ds.append(d)
                            cs.append(c)
                            dys.append(dy + 2)
                ones_m.append(m)
                ones_c.append(c)
        cls_idx.append(
            (
                np.array(ks),
                np.array(ms),
                np.array(ds),
                np.array(cs),
                np.array(dys),
                np.array(ones_m),
                np.array(ones_c),
                ones_mask,
            )
        )

    out = np.zeros((nb, NCLS, 5, 128, 128), np.float32)
    for b in range(nb):
        W3 = flat_col[b, :9].reshape(3, 3)  # [d, c]
        shift = flat_col[b, 9:12]
        k5 = flat_col[b, 12:37].reshape(5, 5)
        sp = W3.T @ shift
        for cls in range(NCLS):
            ks, ms, ds, cs, dys, ones_m, ones_c, ones_mask = cls_idx[cls]
            wvals = W3[ds, cs]
            for dxi in range(5):
                S = np.zeros((128, 128), np.float32)
                S[ks, ms] = (wvals * k5[dys, dxi]).astype(np.float32)
                S[ONES, ones_m] = (sp[ones_c] * (ones_mask @ k5[:, dxi])).astype(
                    np.float32
                )
                out[b, cls, dxi] = S
    return out


def _prep_inputs(im, flat_col):
    im = np.asarray(im, dtype=np.float32)
    stats = _build_stats(flat_col)  # [BS, NCLS, 5, 128, 128]

    im_pad = np.zeros((BS, C, HPAD, FREE), np.float32)
    im_pad[:, :, 2 : 2 + H, 2 : 2 + W] = im

    ones_row = np.zeros((1, FREE), np.float32)
    ones_row[0, 2 : 2 + W] = 1.0

    in_maps = []
    for ci in range(NCORES):
        sl = slice(ci * SPC, (ci + 1) * SPC)
        # [SPC, NCLS, 5, 128k, 128m] -> [128k, SPC*NCLS*5*128m]
        st = np.ascontiguousarray(
            stats[sl].transpose(3, 0, 1, 2, 4).reshape(128, STATW)
        )
        in_maps.append(
            {
                "im_pad": np.ascontiguousarray(im_pad[sl]),
                "stat": st,
                "ones_row": ones_row,
            }
        )
    return in_maps


def _run(im, flat_col, trace=False, **trace_kwargs):
    nc = _get_program()
    in_maps = _prep_inputs(im, flat_col)
    res = run_bass_kernel_spmd(
        nc, in_maps, list(range(NCORES)), trace=trace, **trace_kwargs
    )
    out = np.concatenate([r["out"] for r in res.results], axis=0)
    return out, res


def kernel(im, flat_col):
    out, _ = _run(im, flat_col)
    return out


# revision 5
# speedup vs baseline: 1.5668x; 1.5668x over previous
"""ColWarp (per-sample color warp + shift + depthwise 5x5 conv) on 8 TRN2 cores.

Decomposition: out[c] = conv5x5(sum_d W[d,c]*(im[d]+shift[d]), k) is linear,
so the 3x3 color warp, the per-channel shift, and the 5-tap column conv all
fold into per-sample banded Toeplitz stationary matrices built on host from
flat_col (32x37).  Each output row-tile is 5 PSUM-accumulated fp32r matmuls
(one per kernel column dx, the dx shift expressed as a free-dim offset into a
zero-padded rhs tile).  The shift term rides on a constant ones row at
partition 126 whose stationary coefficient is s'[c] * (sum of dy-valid kernel
taps), which reproduces the zero-padding border behavior exactly.

Data parallel: 4 samples per core, 8 cores, no cross-core communication.
"""

import numpy as np

import concourse.bass as bass
import concourse.mybir as mybir
import concourse.tile as tile
from concourse.bass_utils import run_bass_kernel_spmd

BS, C, H, W = 32, 3, 768, 768
NCORES = 8
SPC = BS // NCORES  # 4 samples per core

ROWS = 38  # output rows per h-tile
WIN = ROWS + 4  # input window rows per channel (42)
NT = -(-H // ROWS)  # 21 h-tiles (last has 8 rows)
KC = 3 * WIN + 1  # matmul contraction: 126 image rows + ones row
ONES = 3 * WIN  # partition index of the ones row (126)
FREE = W + 4  # rhs width incl. 2+2 zero side cols (772)
HPAD = 802  # padded height: 2 + 768 + 32 (t20 window slack)
NCLS = 3  # stationary classes: first / interior / last tile
NSTAT = NCLS * 5  # stationaries per sample
STATW = SPC * NSTAT * 128  # stat tensor free width per core

_nc_cache = {}


def _tile_geom(t):
    a = t * ROWS
    rows = min(ROWS, H - a)
    cls = 0 if t == 0 else (2 if t == NT - 1 else 1)
    return a, rows, cls


def _legalize_waits(nc):
    # This walrus build rejects >1 sync wait per instruction; move extra
    # waits onto same-engine NOPs immediately before (sequencers execute
    # waits in program order, so this is equivalent).
    for f in nc.m.functions:
        for blk in f.blocks:
            out = []
            changed = False
            for inst in blk.instructions:
                si = inst.sync_info
                waits = list(si.on_wait) if si is not None and si.on_wait else []
                if len(waits) > 1:
                    changed = True
                    for j, w in enumerate(waits[:-1]):
                        out.append(
                            mybir.InstNoOp(
                                name=f"{inst.name}-wsplit{j}",
                                engine=inst.engine,
                                ins=[],
                                outs=[],
                                sync_info=mybir.SyncInfo(on_wait=[w], on_update=[]),
                            )
                        )
                    inst.sync_info = mybir.SyncInfo(
                        on_wait=[waits[-1]],
                        on_update=list(si.on_update) if si.on_update else [],
                    )
                out.append(inst)
            if changed:
                blk.instructions = out


def _build_program():
    f32 = mybir.dt.float32
    f32r = mybir.dt.float32r
    nc = bass.Bass()
    im_dram = nc.declare_dram_parameter(
        "im_pad", [SPC, C, HPAD, FREE], f32r, isOutput=False
    )
    stat_dram = nc.declare_dram_parameter("stat", [128, STATW], f32r, isOutput=False)
    ones_dram = nc.declare_dram_parameter("ones_row", [1, FREE], f32r, isOutput=False)
    out_dram = nc.declare_dram_parameter("out", [SPC, C, H, W], f32, isOutput=True)

    with tile.TileContext(nc) as tc:
        with (
            tc.tile_pool(name="stat", bufs=1) as stat_pool,
            tc.tile_pool(name="rhs", bufs=4) as rhs_pool,
            tc.tile_pool(name="outb", bufs=4) as out_pool,
            tc.tile_pool(name="psum", bufs=3, space="PSUM") as psum_pool,
        ):
            stat_t = stat_pool.tile([128, STATW], f32r)
            per_b = NSTAT * 128
            for b in range(SPC):
                nc.sync.dma_start(
                    out=stat_t[:, b * per_b : (b + 1) * per_b],
                    in_=stat_dram[:, b * per_b : (b + 1) * per_b],
                )

            n_tile = 0
            for b in range(SPC):
                for t in range(NT):
                    a, rows, cls = _tile_geom(t)
                    mt = 3 * rows

                    rhs_t = rhs_pool.tile([128, FREE], f32r, tag="rhs")
                    # window rows [a-2, a+40) of the original image =
                    # rows [a, a+42) of the host-padded image
                    nc.gpsimd.dma_start(
                        out=rhs_t[0 : 3 * WIN, :],
                        in_=im_dram[b, :, a : a + WIN, :],
                    )
                    nc.sync.dma_start(out=rhs_t[ONES : ONES + 1, :], in_=ones_dram[:])

                    psum_t = psum_pool.tile([114, W], f32, tag="ps")
                    for cs, nsz in ((0, 512), (512, 256)):
                        for dxi in range(5):
                            idx = (b * NCLS + cls) * 5 + dxi
                            nc.tensor.matmul(
                                psum_t[:mt, cs : cs + nsz],
                                stat_t[:KC, idx * 128 : idx * 128 + mt],
                                rhs_t[:KC, cs + dxi : cs + dxi + nsz],
                                start=(dxi == 0),
                                stop=(dxi == 4),
                            )

                    out_t = out_pool.tile([114, W], f32, tag="ob")
                    if n_tile % 2 == 0:
                        nc.vector.tensor_copy(out=out_t[:mt, :], in_=psum_t[:mt, :])
                    else:
                        nc.scalar.copy(out=out_t[:mt, :], in_=psum_t[:mt, :])
                    nc.gpsimd.dma_start(
                        out=out_dram[b, :, a : a + rows, :],
                        in_=out_t[:mt, :],
                    )
                    n_tile += 1

    _legalize_waits(nc)
    return nc


def _get_program():
    if "nc" not in _nc_cache:
        _nc_cache["nc"] = _build_program()
    return _nc_cache["nc"]


def _build_stats(flat_col):
    """flat_col [BS, 37] float32 -> stats [BS, NCLS, 5, 128, 128] float32."""
    flat_col = np.asarray(flat_col, np.float64)
    nb = flat_col.shape[0]
    geoms = [_tile_geom(t) for t in (0, 1, NT - 1)]

    # index arrays per class (independent of sample and dx)
    cls_idx = []
    for a, rows, cls in geoms:
        base = a - 2
        ks, ms, ds, cs, dys = [], [], [], [], []
        ones_m = []
        ones_c = []
        ones_mask = np.zeros((3 * rows, 5), np.float64)
        for c in range(3):
            for i in range(rows):
                m = c * rows + i
                h = a + i
                for dy in range(-2, 3):
                    h2 = h + dy
                    if 0 <= h2 < H:
                        ones_mask[m, dy + 2] = 1.0
                        for d in range(3):
                            ks.append(d * WIN + (h2 - base))
                            ms.append(m)
                            ds.append(d)
                            cs.append(c)
                            dys.append(dy + 2)
                ones_m.append(m)
                ones_c.append(c)
        cls_idx.append(
            (
                np.array(ks),
                np.array(ms),
                np.array(ds),
                np.array(cs),
                np.array(dys),
                np.array(ones_m),
                np.array(ones_c),
                ones_mask,
            )
        )

    out = np.zeros((nb, NCLS, 5, 128, 128), np.float32)
    for b in range(nb):
        W3 = flat_col[b, :9].reshape(3, 3)  # [d, c]
        shift = flat_col[b, 9:12]
        k5 = flat_col[b, 12:37].reshape(5, 5)
        sp = W3.T @ shift
        for cls in range(NCLS):
            ks, ms, ds, cs, dys, ones_m, ones_c, ones_mask = cls_idx[cls]
            wvals = W3[ds, cs]
            for dxi in range(5):
                S = np.zeros((128, 128), np.float32)
                S[ks, ms] = (wvals * k5[dys, dxi]).astype(np.float32)
                S[ONES, ones_m] = (sp[ones_c] * (ones_mask @ k5[:, dxi])).astype(
                    np.float32
                )
                out[b, cls, dxi] = S
    return out


def _prep_inputs(im, flat_col):
    im = np.asarray(im, dtype=np.float32)
    stats = _build_stats(flat_col)  # [BS, NCLS, 5, 128, 128]

    im_pad = np.zeros((BS, C, HPAD, FREE), np.float32)
    im_pad[:, :, 2 : 2 + H, 2 : 2 + W] = im

    ones_row = np.zeros((1, FREE), np.float32)
    ones_row[0, 2 : 2 + W] = 1.0

    in_maps = []
    for ci in range(NCORES):
        sl = slice(ci * SPC, (ci + 1) * SPC)
        # [SPC, NCLS, 5, 128k, 128m] -> [128k, SPC*NCLS*5*128m]
        st = np.ascontiguousarray(
            stats[sl].transpose(3, 0, 1, 2, 4).reshape(128, STATW)
        )
        in_maps.append(
            {
                "im_pad": np.ascontiguousarray(im_pad[sl]),
                "stat": st,
                "ones_row": ones_row,
            }
        )
    return in_maps


def _run(im, flat_col, trace=False, **trace_kwargs):
    nc = _get_program()
    in_maps = _prep_inputs(im, flat_col)
    res = run_bass_kernel_spmd(
        nc, in_maps, list(range(NCORES)), trace=trace, **trace_kwargs
    )
    out = np.concatenate([r["out"] for r in res.results], axis=0)
    return out, res


def kernel(im, flat_col):
    out, _ = _run(im, flat_col)
    return out
